# revision 4
# baseline (speedup 1.0000x reference)
"""Expert-parallel fp8(e4m3fn) dequant kernel for Trainium2 (8 NeuronCores).

Problem: weight (64, 4096, 1024) uint8 = raw fp8_e4m3fn bit patterns,
scale (64, 4096) fp32. Output (64, 4096, 1024) bf16 =
fp8_decode(weight) * bf16(scale)[..., None].

TRN2's native FP8_EXP4 is NOT OCP e4m3fn (exponent=1111 decodes to
Inf/NaN instead of 256..448), so we decode with integer/fp16 bit
arithmetic instead of the hardware fp8 path:

    B(u16) = x*128 + (x>=128)*16384        # = sign<<15 | mag<<7
    view B as fp16  ->  value = w * 2^-8   (exact, incl. subnormals)
    out_bf16 = fp16(B) * (bf16(scale) * 256)   # exact product, single RNE

which is bitwise identical to jax's  fp8.astype(bf16) * scale.astype(bf16).

Sharding: dim 0 (experts) split 8 ways; zero communication.
Engine split per super-tile [128 partitions x 8192 bytes]:
  ACT:    t1 = x*128            (u8 -> u16)
  GPSIMD: t2 = (x>=128)*16384   (u8 -> u16)
  DVE:    t1 += t2 ; out = fp16(t1) * scale_row -> bf16
"""
import sys

if '/opt/trn_rl_repo' not in sys.path:
    sys.path.insert(0, '/opt/trn_rl_repo')

import numpy as np
import ml_dtypes

E, O, I = 64, 4096, 1024
N_CORES = 8
E_PER = E // N_CORES          # 8 experts per core
R = E_PER * O                 # 32768 rows per core
P = 128                       # partitions
RPP = 8                       # rows per partition per super-tile
G = R // (P * RPP)            # 32 super-tiles per core
FD = RPP * I                  # 8192 bytes free-dim per super-tile

_cache = {}


def _build_nc(repeat=1):
    import concourse.bacc as bacc
    import concourse.mybir as mybir
    from concourse.mybir import AluOpType as A
    from concourse.tile import TileContext

    u8, u16, f32 = mybir.dt.uint8, mybir.dt.uint16, mybir.dt.float32
    bf16, fp16 = mybir.dt.bfloat16, mybir.dt.float16

    nc = bacc.Bacc(trn_type="TRN2", enable_partition_id=False)
    w = nc.dram_tensor("w", [R, I], u8, kind="ExternalInput")
    s = nc.dram_tensor("s", [P, G * RPP], f32, kind="ExternalInput")
    y = nc.dram_tensor("y", [R, I], bf16, kind="ExternalOutput")

    # row r = (g*128 + p)*RPP + j  ->  [g, p, (j i)]
    wv = w.rearrange("(g p j) i -> g p (j i)", p=P, j=RPP)
    yv = y.rearrange("(g p j) i -> g p (j i)", p=P, j=RPP)

    with TileContext(nc) as tc:
        with tc.tile_pool(name="scp", bufs=1) as scp, \
             tc.tile_pool(name="pool", bufs=3) as pool:
            # scale prep: round to bf16 (RNE) then *256, all on DVE
            st0 = scp.tile([P, G * RPP], f32)
            nc.sync.dma_start(st0[:], s[:])
            sbf = scp.tile([P, G * RPP], bf16)
            nc.vector.tensor_copy(sbf[:], st0[:])
            s2 = scp.tile([P, G * RPP], f32)
            nc.vector.tensor_scalar(s2[:], sbf[:], 256.0, None, A.mult)

            for g in [g for _ in range(repeat) for g in range(G)]:
                xt = pool.tile([P, FD], u8, tag="xt")
                nc.sync.dma_start(xt[:], wv[g])
                t1 = pool.tile([P, FD], u16, tag="t1")
                nc.scalar.mul(t1[:], xt[:], 128.0)
                t2 = pool.tile([P, FD], u16, tag="t2")
                nc.gpsimd.tensor_scalar(t2[:], xt[:], 128.0, 16384.0, A.is_ge, A.mult)
                nc.vector.tensor_tensor(t1[:], t1[:], t2[:], A.add)
                ot = pool.tile([P, FD], bf16, tag="ot")
                for j in range(RPP):
                    nc.vector.tensor_scalar(
                        ot[:, j * I:(j + 1) * I],
                        t1[:, j * I:(j + 1) * I].bitcast(fp16),
                        s2[:, g * RPP + j: g * RPP + j + 1], None, A.mult)
                nc.sync.dma_start(yv[g], ot[:])

    nc.compile()
    return nc


def _prep_scale(scale_c: np.ndarray) -> np.ndarray:
    """scale_c: (R,) fp32 for this core -> [P, G*RPP] with
    sp[p, g*RPP+j] = scale_c[(g*128+p)*RPP + j]."""
    return np.ascontiguousarray(
        scale_c.reshape(G, P, RPP).transpose(1, 0, 2).reshape(P, G * RPP))


def kernel(weight: np.ndarray, scale: np.ndarray) -> np.ndarray:
    from concourse import bass_utils

    weight = np.asarray(weight)
    scale = np.asarray(scale)
    assert weight.shape == (E, O, I) and scale.shape == (E, O)
    w8 = weight.view(np.uint8) if weight.dtype != np.uint8 else weight
    sc = scale.astype(np.float32, copy=False)

    if "nc" not in _cache:
        _cache["nc"] = _build_nc()
    nc = _cache["nc"]

    in_maps = []
    for c in range(N_CORES):
        wc = np.ascontiguousarray(w8[c * E_PER:(c + 1) * E_PER].reshape(R, I))
        scc = _prep_scale(np.ascontiguousarray(
            sc[c * E_PER:(c + 1) * E_PER].reshape(R)))
        in_maps.append({"w": wc, "s": scc})

    res = bass_utils.run_bass_kernel_spmd(nc, in_maps, core_ids=list(range(N_CORES)))
    out = np.empty((E, O, I), dtype=ml_dtypes.bfloat16)
    for c in range(N_CORES):
        yc = np.asarray(res.results[c]["y"]).reshape(E_PER, O, I)
        out[c * E_PER:(c + 1) * E_PER] = yc
    return out


# revision 8
# speedup vs baseline: 11.8825x; 11.8825x over previous
"""Expert-parallel fp8(e4m3fn) dequant kernel for Trainium2 (8 NeuronCores).

Problem: weight (64, 4096, 1024) uint8 = raw fp8_e4m3fn bit patterns,
scale (64, 4096) fp32. Output (64, 4096, 1024) bf16 =
fp8_decode(weight) * bf16(scale)[..., None].

TRN2's native FP8_EXP4 is NOT OCP e4m3fn (exponent=1111 decodes to
Inf/NaN instead of 256..448), so we decode with integer/fp16 bit
arithmetic instead of the hardware fp8 path:

    B(u16) = x*128 + (x>=128)*16384        # = sign<<15 | mag<<7
    view B as fp16  ->  value = w * 2^-8   (exact, incl. subnormals)
    out_bf16 = fp16(B) * (bf16(scale) * 256)   # exact product, single RNE

which is bitwise identical to jax's  fp8.astype(bf16) * scale.astype(bf16).

Sharding: dim 0 (experts) split 8 ways; zero communication.
Engine split per super-tile [128 partitions x 8192 bytes]:
  ACT:    t1 = x*128            (u8 -> u16)
  GPSIMD: t2 = (x>=128)*16384   (u8 -> u16)
  DVE:    t1 += t2 ; out = fp16(t1) * scale_row -> bf16
"""
import sys

if '/opt/trn_rl_repo' not in sys.path:
    sys.path.insert(0, '/opt/trn_rl_repo')

import numpy as np
import ml_dtypes

E, O, I = 64, 4096, 1024
N_CORES = 8
E_PER = E // N_CORES          # 8 experts per core
R = E_PER * O                 # 32768 rows per core
P = 128                       # partitions
RPP = 8                       # rows per partition per super-tile
G = R // (P * RPP)            # 32 super-tiles per core
FD = RPP * I                  # 8192 bytes free-dim per super-tile

_cache = {}


def _build_nc(repeat=1, stages=("act", "gp", "add", "mul"), bufs=3):
    import concourse.bacc as bacc
    import concourse.mybir as mybir
    from concourse.mybir import AluOpType as A
    from concourse.tile import TileContext

    u8, u16, f32 = mybir.dt.uint8, mybir.dt.uint16, mybir.dt.float32
    bf16, fp16 = mybir.dt.bfloat16, mybir.dt.float16

    nc = bacc.Bacc(trn_type="TRN2", enable_partition_id=False)
    w = nc.dram_tensor("w", [R, I], u8, kind="ExternalInput")
    s = nc.dram_tensor("s", [P, G * RPP], f32, kind="ExternalInput")
    y = nc.dram_tensor("y", [R, I], bf16, kind="ExternalOutput")

    # row r = (g*128 + p)*RPP + j  ->  [g, p, (j i)]
    wv = w.rearrange("(g p j) i -> g p (j i)", p=P, j=RPP)
    yv = y.rearrange("(g p j) i -> g p (j i)", p=P, j=RPP)

    with TileContext(nc) as tc:
        with tc.tile_pool(name="scp", bufs=1) as scp, \
             tc.tile_pool(name="pool", bufs=bufs) as pool:
            # scale prep: round to bf16 (RNE) then *256, all on DVE
            st0 = scp.tile([P, G * RPP], f32)
            nc.sync.dma_start(st0[:], s[:])
            sbf = scp.tile([P, G * RPP], bf16)
            nc.vector.tensor_copy(sbf[:], st0[:])
            s2 = scp.tile([P, G * RPP], f32)
            nc.vector.tensor_scalar(s2[:], sbf[:], 256.0, None, A.mult)

            for g in [g for _ in range(repeat) for g in range(G)]:
                xt = pool.tile([P, FD], u8, tag="xt")
                nc.sync.dma_start(xt[:], wv[g])
                t1 = pool.tile([P, FD], u16, tag="t1")
                if "act" in stages:
                    nc.scalar.mul(t1[:], xt[:], 128.0)
                elif "dve1" in stages:
                    nc.vector.tensor_scalar(t1[:], xt[:], 128.0, None, A.mult)
                t2 = pool.tile([P, FD], u16, tag="t2")
                if "gp" in stages:
                    nc.gpsimd.tensor_scalar(t2[:], xt[:], 128.0, 16384.0, A.is_ge, A.mult)
                elif "dve2" in stages:
                    nc.vector.tensor_scalar(t2[:], xt[:], 128.0, 16384.0, A.is_ge, A.mult)
                if "add" in stages:
                    nc.vector.tensor_tensor(t1[:], t1[:], t2[:], A.add)
                ot = pool.tile([P, FD], bf16, tag="ot")
                if "mul" in stages:
                    for j in range(RPP):
                        nc.vector.tensor_scalar(
                            ot[:, j * I:(j + 1) * I],
                            t1[:, j * I:(j + 1) * I].bitcast(fp16),
                            s2[:, g * RPP + j: g * RPP + j + 1], None, A.mult)
                elif "mulact" in stages:
                    import concourse.mybir as _mb
                    for j in range(RPP):
                        nc.scalar.activation(
                            ot[:, j * I:(j + 1) * I],
                            t1[:, j * I:(j + 1) * I].bitcast(fp16),
                            _mb.ActivationFunctionType.Copy,
                            scale=s2[:, g * RPP + j: g * RPP + j + 1])
                elif "mulcopy" in stages:
                    nc.vector.tensor_copy(ot[:], t1[:].bitcast(bf16))
                if not any(s in stages for s in ("mul", "mulact", "mulcopy")):
                    # DMA-traffic-only variant: ship xt's bytes out twice
                    xb = xt[:].bitcast(bf16)
                    nc.sync.dma_start(yv[g][:, :FD // 2], xb)
                    nc.sync.dma_start(yv[g][:, FD // 2:], xb)
                else:
                    nc.sync.dma_start(yv[g], ot[:])

    nc.compile()
    return nc


def _prep_scale(scale_c: np.ndarray) -> np.ndarray:
    """scale_c: (R,) fp32 for this core -> [P, G*RPP] with
    sp[p, g*RPP+j] = scale_c[(g*128+p)*RPP + j]."""
    return np.ascontiguousarray(
        scale_c.reshape(G, P, RPP).transpose(1, 0, 2).reshape(P, G * RPP))


def kernel(weight: np.ndarray, scale: np.ndarray) -> np.ndarray:
    from concourse import bass_utils

    weight = np.asarray(weight)
    scale = np.asarray(scale)
    assert weight.shape == (E, O, I) and scale.shape == (E, O)
    w8 = weight.view(np.uint8) if weight.dtype != np.uint8 else weight
    sc = scale.astype(np.float32, copy=False)

    if "nc" not in _cache:
        _cache["nc"] = _build_nc()
    nc = _cache["nc"]

    in_maps = []
    for c in range(N_CORES):
        wc = np.ascontiguousarray(w8[c * E_PER:(c + 1) * E_PER].reshape(R, I))
        scc = _prep_scale(np.ascontiguousarray(
            sc[c * E_PER:(c + 1) * E_PER].reshape(R)))
        in_maps.append({"w": wc, "s": scc})

    res = bass_utils.run_bass_kernel_spmd(nc, in_maps, core_ids=list(range(N_CORES)))
    out = np.empty((E, O, I), dtype=ml_dtypes.bfloat16)
    for c in range(N_CORES):
        yc = np.asarray(res.results[c]["y"]).reshape(E_PER, O, I)
        out[c * E_PER:(c + 1) * E_PER] = yc
    return out
